# revision 1
# baseline (speedup 1.0000x reference)
"""ConceptNet encoder kernel for 8 Trainium2 NeuronCores (Bass/Tile).

Reference computation:
    emb    = table[tok]                      # [1024, 256]
    logits = emb @ table.T                   # [1024, 100000]
    idx    = top16(softmax(logits))          # softmax monotonic -> top16(logits)
    h      = table[idx]                      # [1024, 16, 256]
    e      = tanh(h @ a) @ b                 # [1024, 16]
    out    = softmax(e) @ h                  # [1024, 256]

Distribution: vocab sharded 8 ways. The similarity matmul runs in
float32r (~bf16 speed, ~12-bit mantissa inputs, fp32 accumulate).
Selection packs (quantized value, slot) into one uint32 key per logit
(scalar-engine quantize + one DVE shift-or pass; a few chunks take a
scalar-shift + gpsimd-add path instead to offload the DVE), takes
per-chunk top-8 with a single MAX8 (no FIND_INDEX8, no index plane),
and AllToAll's one key plane per chunk-group (4 pipelined collectives
overlapped with compute). The vocab-shard strips stream through a ring
of SBUF tiles (chunk-group-outer loop), so there is no bulk-load
startup stall. Each core then merges a top-20 candidate pool for its
own 128 tokens, re-scores the pool exactly in fp32 (rescue), and runs
masked-softmax attention over the pool so exactly the true top-16 get
weight.

kernel(**inputs) takes FULL unsharded inputs, returns FULL [4,256,256] output.
Self-contained: hardcodes all shapes; imports only the system concourse repo.
"""
import os
import sys

if "/opt/trn_rl_repo" not in sys.path:
    sys.path.insert(0, "/opt/trn_rl_repo")

import numpy as np

import concourse.bass as bass
import concourse.bacc as bacc
import concourse.mybir as mybir
import concourse.tile as tile
from concourse import bass_utils
from concourse.masks import make_identity

DT = mybir.dt
ALU = mybir.AluOpType
ACT = mybir.ActivationFunctionType

B, L, V, E, TOPK = 4, 256, 100000, 256, 16
NCORES = 8
NTOK = B * L                 # 1024
TPC = NTOK // NCORES         # 128 tokens per core (merge/attention shard)
VS = V // NCORES             # 12500 vocab rows per core
P = 128
NEG = -3.0e38

CW = 1024                    # similarity chunk width (2 PSUM banks)
CHUNKS = []
_off = 0
while _off < VS:
    CHUNKS.append((_off, min(CW, VS - _off)))
    _off += CW
NCHUNK = len(CHUNKS)         # 13 (12x1024 + 212)
PARTS = [[0, 1, 2], [3, 4, 5], [6, 7, 8], [9, 10, 11, 12]]  # j = 3*part + jl
GP_CHUNKS = {0, 1, 2}        # gpsimd-assisted pack: part 0 only (collectives
                             # block the gpsimd queue once they are issued)
RING = 10                    # streamed tabT strip ring depth (per kb)
NCP = 32                     # candidate slots per (core, part): <=4 chunks x 8
KP = 20                      # rescue pool size per token
KPAD = 24                    # padded pool for max8 rounds
NGR = KP // 4                # attention 512-wide groups
QSCALE = 1536.0              # logit quantizer scale
QBIAS = 3456.0               # makes qi positive (logits in [-2.25, 2])
KEYSHIFT = 11                # slot bits
KEYBASE = 1 << 30            # keeps key bit patterns in normal-float range
MERGEW = 4 * NCORES * NCP    # 1024

_BUILD_CACHE = {}
LAST_RESULTS = None


def _round12(x):
    """Round fp32 to 12 explicit mantissa bits (round half even)."""
    u = np.ascontiguousarray(x, dtype=np.float32).view(np.uint32)
    shift = np.uint32(11)
    mask = np.uint32((1 << 11) - 1)
    half = np.uint32(1 << 10)
    frac = u & mask
    u2 = u & ~mask
    rnd = (frac > half) | ((frac == half) & (((u2 >> shift) & np.uint32(1)) == 1))
    u2 = u2 + (rnd.astype(np.uint32) << shift)
    return u2.view(np.float32)


def _build():
    nc = bacc.Bacc("TRN2", target_bir_lowering=False, debug=False,
                   enable_asserts=True, num_devices=NCORES)

    tokidx = nc.dram_tensor("tokidx", [NTOK, 1], DT.int32, kind="ExternalInput").ap()
    tok_own = nc.dram_tensor("tok_own", [TPC, 1], DT.int32, kind="ExternalInput").ap()
    table = nc.dram_tensor("table", [V, E], DT.float32, kind="ExternalInput").ap()
    tabTr = nc.dram_tensor("tabTr", [E, VS], DT.float32r, kind="ExternalInput").ap()
    amat = nc.dram_tensor("amat", [E, E], DT.float32, kind="ExternalInput").ap()
    bvec = nc.dram_tensor("bvec", [E, 1], DT.float32, kind="ExternalInput").ap()
    out = nc.dram_tensor("out", [TPC, E], DT.float32, kind="ExternalOutput").ap()

    with tile.TileContext(nc) as tc:
        with tc.tile_pool(name="const", bufs=1) as cpool, \
             tc.tile_pool(name="big", bufs=1) as big, \
             tc.tile_pool(name="work", bufs=2) as work, \
             tc.tile_pool(name="ps_chunk", bufs=3, space="PSUM") as ps_chunk, \
             tc.tile_pool(name="ps_tr", bufs=2, space="PSUM") as ps_tr, \
             tc.tile_pool(name="dram", bufs=1, space="DRAM") as dram:

            # ---------------- constants ----------------
            ident = cpool.tile([P, P], DT.float32, tag="ident")
            make_identity(nc, ident)

            iotaK = cpool.tile([P, CW], DT.uint32, tag="iotaK")
            nc.gpsimd.iota(iotaK, pattern=[[1, CW]], base=KEYBASE,
                           channel_multiplier=0)

            def const_col(name, val):
                t = cpool.tile([P, 1], DT.uint32, tag=name, name=name)
                nc.gpsimd.iota(t, pattern=[[0, 1]], base=val, channel_multiplier=0)
                return t

            c_shift = const_col("c_shift", KEYSHIFT)
            c_slotmask = const_col("c_slotmask", (1 << KEYSHIFT) - 1)
            c_8 = const_col("c_8", 8)
            c_5 = const_col("c_5", 5)
            c_7 = const_col("c_7", 7)
            c_3 = const_col("c_3", 3)

            # ---------------- emb gather + f32r transpose ----------------
            embT = [[big.tile([P, P], DT.float32r, tag=f"embT{kb}_{m}",
                              name=f"embT{kb}_{m}")
                     for m in range(NCORES)] for kb in range(2)]
            for m in range(NCORES):
                ti = work.tile([P, 1], DT.int32, tag="ti")
                nc.sync.dma_start(out=ti, in_=tokidx[m * P:(m + 1) * P, :])
                em = work.tile([P, E], DT.float32, tag="em")
                nc.gpsimd.indirect_dma_start(
                    out=em, out_offset=None, in_=table,
                    in_offset=bass.IndirectOffsetOnAxis(ap=ti[:, :], axis=0))
                for kb in range(2):
                    pt = ps_tr.tile([P, P], DT.float32, tag="tr")
                    nc.tensor.transpose(out=pt, in_=em[:, kb * P:(kb + 1) * P],
                                        identity=ident)
                    nc.vector.tensor_copy(embT[kb][m], pt)

            # own-token embeddings (fp32, for exact rescue dots)
            ti_own = cpool.tile([P, 1], DT.int32, tag="ti_own")
            nc.sync.dma_start(out=ti_own, in_=tok_own)
            emb_own = cpool.tile([P, E], DT.float32, tag="emb_own")
            nc.gpsimd.indirect_dma_start(
                out=emb_own, out_offset=None, in_=table,
                in_offset=bass.IndirectOffsetOnAxis(ap=ti_own[:, :], axis=0))

            # ---------------- small weights ----------------
            a_sb = []
            for kb in range(2):
                t = cpool.tile([P, E], DT.float32, tag=f"a{kb}", name=f"a{kb}")
                nc.sync.dma_start(out=t, in_=amat[kb * P:(kb + 1) * P, :])
                a_sb.append(t)
            a_r = []
            for kb in range(2):
                t = cpool.tile([P, E], DT.float32r, tag=f"ar{kb}", name=f"ar{kb}")
                nc.vector.tensor_copy(t, a_sb[kb])
                a_r.append(t)
            b_sb = []
            for kb in range(2):
                t = cpool.tile([P, 1], DT.float32, tag=f"b{kb}", name=f"b{kb}")
                nc.sync.dma_start(out=t, in_=bvec[kb * P:(kb + 1) * P, :])
                b_sb.append(t)
            b_r = []
            for kb in range(2):
                t = cpool.tile([P, 1], DT.float32r, tag=f"br{kb}", name=f"br{kb}")
                nc.vector.tensor_copy(t, b_sb[kb])
                b_r.append(t)

            # ---------------- streamed tabT strips ----------------
            strip = {}

            def issue_strips(part):
                for j in PARTS[part]:
                    off, w = CHUNKS[j]
                    for kb in range(2):
                        t = big.tile([P, w], DT.float32r, tag=f"tt{kb}_{j % RING}",
                                     name=f"tt{kb}_{j % RING}")
                        nc.sync.dma_start(out=t, in_=tabTr[kb * P:(kb + 1) * P,
                                                           off:off + w])
                        strip[(kb, j)] = t

            issue_strips(0)
            issue_strips(1)

            vals = cpool.tile([P, MERGEW], DT.float32, tag="vals")

            def load_vals(part):
                # vals[p, part*256 + c*32 + s] = agg[part][(c, p, s)]
                agg_v = agg[part][:, :].rearrange("(c p s) o -> c p (s o)",
                                                  c=NCORES, p=TPC)
                for c in range(NCORES):
                    o = part * NCORES * NCP + c * NCP
                    nc.gpsimd.dma_start(out=vals[:, o:o + NCP], in_=agg_v[c])

            # ---------------- a2a bounce buffers ----------------
            bounce = [dram.tile([NCORES, TPC, NCP], DT.float32, tag=f"bounce{p}",
                                name=f"bounce{p}")
                      for p in range(4)]
            agg = [dram.tile([NCORES * TPC * NCP, 1], DT.float32, tag=f"agg{p}",
                             name=f"agg{p}")
                   for p in range(4)]
            scd = dram.tile([1, TPC * KP], DT.float32, tag="scd")

            # ---------------- similarity + packed per-chunk top-8 --------
            for part in range(4):
                if part + 2 < 4:
                    issue_strips(part + 2)
                pjs = PARTS[part]
                for m in range(NCORES):
                    if m == 1 and part >= 1:
                        nc.gpsimd.collective_compute(
                            "AllToAll", ALU.bypass,
                            replica_groups=[list(range(NCORES))],
                            ins=[bounce[part - 1][:, :, :].opt()],
                            outs=[agg[part - 1][:, :].opt()],
                        )
                    if m == 5 and part >= 2:
                        load_vals(part - 2)
                    cv = work.tile([P, NCP], DT.float32, tag="cv")
                    if len(pjs) < 4:
                        nc.vector.memset(cv[:, len(pjs) * 8:], 0.0)
                    for jl, j in enumerate(pjs):
                        off, w = CHUNKS[j]
                        ps = ps_chunk.tile([P, CW], DT.float32, tag="chunk")
                        for kb in range(2):
                            for h in range((w + 511) // 512):
                                hw = min(512, w - h * 512)
                                nc.tensor.matmul(
                                    ps[:, h * 512:h * 512 + hw],
                                    embT[kb][m],
                                    strip[(kb, j)][:, h * 512:h * 512 + hw],
                                    start=(kb == 0), stop=(kb == 1))
                        keys = work.tile([P, CW], DT.uint32, tag="keys", bufs=4)
                        if j in GP_CHUNKS:
                            # scalar: quantize + exact *2048 shift; gpsimd: +iota
                            q1 = work.tile([P, CW], DT.int32, tag="q1", bufs=3)
                            nc.scalar.activation(q1[:, :w], ps[:, :w],
                                                 ACT.Copy, scale=QSCALE,
                                                 bias=QBIAS)
                            nc.scalar.activation(keys[:, :w].bitcast(DT.int32),
                                                 q1[:, :w], ACT.Copy,
                                                 scale=float(1 << KEYSHIFT))
                            nc.gpsimd.tensor_tensor(keys[:, :w], keys[:, :w],
                                                    iotaK[:, :w], op=ALU.add)
                        else:
                            # quantize logits -> int (scalar engine reads PSUM)
                            nc.scalar.activation(keys[:, :w].bitcast(DT.int32),
                                                 ps[:, :w], ACT.Copy,
                                                 scale=QSCALE, bias=QBIAS)
                            # key = (qi << 11) | slot | 2^30  (one DVE pass)
                            nc.vector.scalar_tensor_tensor(
                                keys[:, :w], keys[:, :w], c_shift[:, :],
                                iotaK[:, :w],
                                op0=ALU.logical_shift_left, op1=ALU.bitwise_or)
                        nc.vector.max(out=cv[:, jl * 8:(jl + 1) * 8],
                                      in_=keys[:, :w].bitcast(DT.float32))
                    nc.sync.dma_start(out=bounce[part][m, :, :], in_=cv)

            nc.gpsimd.collective_compute(
                "AllToAll", ALU.bypass,
                replica_groups=[list(range(NCORES))],
                ins=[bounce[3][:, :, :].opt()],
                outs=[agg[3][:, :].opt()],
            )

            load_vals(2)
            load_vals(3)

            # ---------------- merge: top-20 keys + positions -------------
            wk = cpool.tile([P, KPAD], DT.float32, tag="wk")
            wp = cpool.tile([P, KPAD], DT.uint32, tag="wp")
            vals2 = cpool.tile([P, MERGEW], DT.float32, tag="vals2")
            vals3 = cpool.tile([P, MERGEW], DT.float32, tag="vals3")

            # ---------------- decode global vocab indices ----------------
            # pos = part*256 + c*32 + jl*8 + r ; key = (qi<<11)|slot|2^30
            slot = cpool.tile([P, KPAD], DT.uint32, tag="slot", name="slot")
            prt = cpool.tile([P, KPAD], DT.uint32, tag="prt", name="prt")
            csrc = cpool.tile([P, KPAD], DT.uint32, tag="csrc", name="csrc")
            jl = cpool.tile([P, KPAD], DT.uint32, tag="jl", name="jl")
            gidx = cpool.tile([P, KPAD], DT.uint32, tag="gidx", name="gidx")
            t2 = cpool.tile([P, KPAD], DT.uint32, tag="t2", name="t2")
            hk = [cpool.tile([P, E], DT.float32, tag=f"h{k}", name=f"h{k}")
                  for k in range(KP)]

            def decode_and_gather(g0, g1):
                """Decode candidate slots [g0,g1) and launch their h gathers."""
                gs = slice(g0, g1)
                nc.vector.tensor_scalar(slot[:, gs], wk[:, gs].bitcast(DT.uint32),
                                        c_slotmask[:, :], None,
                                        op0=ALU.bitwise_and)
                nc.vector.tensor_scalar(prt[:, gs], wp[:, gs], c_8[:, :], None,
                                        op0=ALU.logical_shift_right)
                nc.vector.tensor_scalar(csrc[:, gs], wp[:, gs], c_5[:, :], None,
                                        op0=ALU.logical_shift_right)
                nc.vector.tensor_scalar(csrc[:, gs], csrc[:, gs], c_7[:, :], None,
                                        op0=ALU.bitwise_and)
                nc.vector.tensor_scalar(jl[:, gs], wp[:, gs], c_3[:, :], None,
                                        op0=ALU.logical_shift_right)
                nc.vector.tensor_scalar(jl[:, gs], jl[:, gs], c_3[:, :], None,
                                        op0=ALU.bitwise_and)
                # gidx = csrc*12500 + (3*part + jl)*1024 + slot (< 2^24: fp-exact)
                nc.vector.tensor_scalar(gidx[:, gs], csrc[:, gs], float(VS),
                                        None, op0=ALU.mult)
                nc.vector.tensor_scalar(t2[:, gs], prt[:, gs], 3.0 * CW, None,
                                        op0=ALU.mult)
                nc.vector.tensor_tensor(gidx[:, gs], gidx[:, gs], t2[:, gs],
                                        op=ALU.add)
                nc.vector.tensor_scalar(t2[:, gs], jl[:, gs], float(CW), None,
                                        op0=ALU.mult)
                nc.vector.tensor_tensor(gidx[:, gs], gidx[:, gs], t2[:, gs],
                                        op=ALU.add)
                nc.vector.tensor_tensor(gidx[:, gs], gidx[:, gs], slot[:, gs],
                                        op=ALU.add)
                for k in range(g0, min(g1, KP)):
                    nc.gpsimd.indirect_dma_start(
                        out=hk[k], out_offset=None, in_=table,
                        in_offset=bass.IndirectOffsetOnAxis(
                            ap=gidx[:, :].bitcast(DT.int32)[:, k:k + 1], axis=0))

            nc.vector.max(out=wk[:, 0:8], in_=vals)
            nc.vector.max_index(out=wp[:, 0:8], in_max=wk[:, 0:8], in_values=vals)
            nc.vector.match_replace(out=vals2, in_to_replace=wk[:, 0:8],
                                    in_values=vals, imm_value=0.0)
            decode_and_gather(0, 8)
            nc.vector.max(out=wk[:, 8:16], in_=vals2)
            nc.vector.max_index(out=wp[:, 8:16], in_max=wk[:, 8:16], in_values=vals2)
            nc.vector.match_replace(out=vals3, in_to_replace=wk[:, 8:16],
                                    in_values=vals2, imm_value=0.0)
            decode_and_gather(8, 16)
            nc.vector.max(out=wk[:, 16:24], in_=vals3)
            nc.vector.max_index(out=wp[:, 16:24], in_max=wk[:, 16:24], in_values=vals3)
            decode_and_gather(16, KP)

            d = cpool.tile([P, KPAD], DT.float32, tag="d")
            nc.vector.memset(d[:, KP:], NEG)
            prod = cpool.tile([P, E], DT.float32, tag="prod")
            for k in range(KP):
                nc.vector.scalar_tensor_tensor(
                    prod, hk[k], 1.0, emb_own,
                    op0=ALU.mult, op1=ALU.mult, accum_out=d[:, k:k + 1])

            # 16th largest exact dot -> threshold mask
            t8a = cpool.tile([P, 8], DT.float32, tag="t8a")
            t8b = cpool.tile([P, 8], DT.float32, tag="t8b")
            d2 = cpool.tile([P, KPAD], DT.float32, tag="d2")
            nc.vector.max(out=t8a, in_=d)
            nc.vector.match_replace(out=d2, in_to_replace=t8a, in_values=d,
                                    imm_value=NEG)
            nc.vector.max(out=t8b, in_=d2)
            # maskp = (1[d >= thr16] - 1) * 1e9   (0 for kept, -1e9 for dropped)
            maskp = cpool.tile([P, KP], DT.float32, tag="maskp")
            nc.vector.tensor_scalar(maskp, d[:, :KP], t8b[:, 7:8], None,
                                    op0=ALU.is_ge)
            nc.vector.tensor_scalar(maskp, maskp, -1.0, 1.0e9,
                                    op0=ALU.add, op1=ALU.mult)

            # ---------------- attention over the 20-candidate pool -------
            # hT chunks: n = k*128 + t, grouped 4 k's per 512-wide chunk
            for g in range(NGR):
                hTs = [work.tile([P, 512], DT.float32r, tag=f"hTs{kb}",
                                 name=f"hTs{kb}")
                       for kb in range(2)]
                for kk in range(4):
                    k = g * 4 + kk
                    for kb in range(2):
                        pt = ps_tr.tile([P, P], DT.float32, tag="tr")
                        nc.tensor.transpose(out=pt,
                                            in_=hk[k][:, kb * P:(kb + 1) * P],
                                            identity=ident)
                        nc.vector.tensor_copy(hTs[kb][:, kk * P:(kk + 1) * P], pt)
                tanhTs = [work.tile([P, 512], DT.float32r, tag=f"tanhTs{eb}",
                                    name=f"tanhTs{eb}")
                          for eb in range(2)]
                for eb in range(2):
                    pta = ps_chunk.tile([P, 512], DT.float32, tag="chunk", name="pta")
                    for kb in range(2):
                        nc.tensor.matmul(pta, a_r[kb][:, eb * P:(eb + 1) * P],
                                         hTs[kb], start=(kb == 0), stop=(kb == 1))
                    nc.scalar.activation(tanhTs[eb], pta, ACT.Tanh)
                psc = ps_chunk.tile([1, 512], DT.float32, tag="chunk", name="psc")
                for eb in range(2):
                    nc.tensor.matmul(psc, b_r[eb], tanhTs[eb],
                                     start=(eb == 0), stop=(eb == 1))
                scs = work.tile([1, 512], DT.float32, tag="scs")
                nc.vector.tensor_copy(scs, psc)
                nc.sync.dma_start(out=scd[:, g * 512:(g + 1) * 512], in_=scs)

            # scores [t, k] <- scd[k*128 + t]
            sct = cpool.tile([P, KP], DT.float32, tag="sct")
            for g in range(NGR):
                nc.sync.dma_start(
                    out=sct[:, g * 4:(g + 1) * 4],
                    in_=scd[:, g * 512:(g + 1) * 512].rearrange(
                        "o (k t) -> (o t) k", t=TPC))

            # masked softmax over k
            nc.vector.tensor_tensor(sct, sct, maskp, op=ALU.add)
            mx = cpool.tile([P, 1], DT.float32, tag="mx")
            nc.vector.reduce_max(mx, sct, axis=mybir.AxisListType.X)
            negmx = cpool.tile([P, 1], DT.float32, tag="negmx")
            nc.vector.tensor_scalar(negmx, mx, -1.0, None, op0=ALU.mult)
            ex = cpool.tile([P, KP], DT.float32, tag="ex")
            nc.scalar.activation(ex, sct, ACT.Exp, bias=negmx[:, :], scale=1.0)
            sm = cpool.tile([P, 1], DT.float32, tag="sm")
            nc.vector.reduce_sum(sm, ex, axis=mybir.AxisListType.X)
            rc = cpool.tile([P, 1], DT.float32, tag="rc")
            nc.vector.reciprocal(rc, sm)
            att = cpool.tile([P, KP], DT.float32, tag="att_w")
            nc.vector.tensor_scalar(att, ex, rc[:, :], None, op0=ALU.mult)

            # out[t, e] = sum_k att[t,k] * h[t,k,e]
            acc = cpool.tile([P, E], DT.float32, tag="acc")
            accB = cpool.tile([P, E], DT.float32, tag="accB")
            term = cpool.tile([P, E], DT.float32, tag="term", bufs=3)
            nc.vector.memset(acc, 0.0)
            nc.vector.memset(accB, 0.0)
            for k in range(KP):
                if k % 2 == 0:
                    nc.vector.scalar_tensor_tensor(
                        acc, hk[k], att[:, k:k + 1], acc,
                        op0=ALU.mult, op1=ALU.add)
                else:
                    tk = cpool.tile([P, E], DT.float32, tag="term", bufs=3,
                                    name="tk")
                    nc.scalar.activation(tk, hk[k], ACT.Copy,
                                         scale=att[:, k:k + 1])
                    nc.vector.tensor_tensor(accB, accB, tk, op=ALU.add)
            nc.vector.tensor_tensor(acc, acc, accB, op=ALU.add)
            nc.sync.dma_start(out=out, in_=acc)

    nc.compile()
    return nc


def get_nc():
    if "v3" not in _BUILD_CACHE:
        _BUILD_CACHE["v3"] = _build()
    return _BUILD_CACHE["v3"]


def kernel(conceptnet_text_vec, table, a, b, topk=16, **_ignored):
    global LAST_RESULTS
    assert int(topk) == TOPK
    tok = np.asarray(conceptnet_text_vec).reshape(NTOK, 1).astype(np.int32)
    table = np.ascontiguousarray(np.asarray(table, dtype=np.float32))
    a = np.ascontiguousarray(np.asarray(a, dtype=np.float32))
    b = np.ascontiguousarray(np.asarray(b, dtype=np.float32)).reshape(E, 1)
    tabT_r = _round12(np.ascontiguousarray(table.T))   # [E, V], f32r-rounded

    nc = get_nc()
    in_maps = []
    for c in range(NCORES):
        in_maps.append({
            "tokidx": tok,
            "tok_own": np.ascontiguousarray(tok[c * TPC:(c + 1) * TPC]),
            "table": table,
            "tabTr": np.ascontiguousarray(tabT_r[:, c * VS:(c + 1) * VS]),
            "amat": a,
            "bvec": b,
        })
    trace = bool(int(os.environ.get("CN_TRACE", "0")))
    res = bass_utils.run_bass_kernel_spmd(nc, in_maps, core_ids=list(range(NCORES)),
                                          trace=trace)
    LAST_RESULTS = res
    outp = np.concatenate([res.results[c]["out"] for c in range(NCORES)], axis=0)
    return outp.reshape(B, L, E)



# revision 4
# speedup vs baseline: 1.4427x; 1.4427x over previous
"""ConceptNet encoder kernel for 8 Trainium2 NeuronCores (Bass/Tile).

Reference computation:
    emb    = table[tok]                      # [1024, 256]
    logits = emb @ table.T                   # [1024, 100000]
    idx    = top16(softmax(logits))          # softmax monotonic -> top16(logits)
    h      = table[idx]                      # [1024, 16, 256]
    e      = tanh(h @ a) @ b                 # [1024, 16]
    out    = softmax(e) @ h                  # [1024, 256]

Distribution: vocab sharded 8 ways (12500 rows/core), all 1024 tokens
scored on every core, token-sharded merge/attention (128 tokens/core).

The similarity matmul runs in bf16 (logit err sigma ~1e-4, far below
top-16 gaps ~8e-4). Selection: the scalar engine quantizes each PSUM
chunk straight into the HIGH fp16 halves of pre-iota'd uint32 words
(one strided activation per chunk) so each word is
(fp16(QSCALE*logit+QBIAS) << 16) | slot16 — totally ordered as fp32
with the slot as tiebreak.  One DVE MAX8 per 4096-wide group then
yields the top-8 (value+slot packed) with zero extra passes.  Four
AllToAll's (one per vocab group, 8 cands each) are pipelined with
compute.  Each core merges a 256-candidate pool for its own 128
tokens, takes top-20, re-scores the pool exactly in fp32 (rescue),
and runs masked-softmax attention so exactly the true top-16 get
weight.

kernel(**inputs) takes FULL unsharded inputs, returns FULL [4,256,256] output.
Self-contained: hardcodes all shapes; imports only the system concourse repo.
"""
import os
import sys

if "/opt/trn_rl_repo" not in sys.path:
    sys.path.insert(0, "/opt/trn_rl_repo")

import numpy as np
import ml_dtypes

import concourse.bass as bass
import concourse.bacc as bacc
import concourse.mybir as mybir
import concourse.tile as tile
from concourse import bass_utils
from concourse.masks import make_identity

DT = mybir.dt
ALU = mybir.AluOpType
ACT = mybir.ActivationFunctionType

B, L, V, E, TOPK = 4, 256, 100000, 256, 16
NCORES = 8
NTOK = B * L                 # 1024
TPC = NTOK // NCORES         # 128 tokens per core (merge/attention shard)
VS = V // NCORES             # 12500 vocab rows per core
P = 128
NEG = -3.0e38

CW = 2048                    # psum chunk width (4 banks)
# chunk (offset, width) within the shard; groups of chunks share one a2a
CHUNKS = [(0, 2048), (2048, 2048), (4096, 2048), (6144, 2048),
          (8192, 2048), (10240, 2048), (12288, 212)]
GROUPS = [[0, 1], [2, 3], [4, 5], [6]]
GRP_OFF = [0, 4096, 8192, 12288]
GRP_W = [4096, 4096, 4096, 212]
NGRP = 4
KEYW = 4096                  # keys tile width
NCP = 8                      # candidates per (core, group)
MERGEW = NGRP * NCORES * NCP  # 256
KP = 20                      # rescue pool size per token
KPAD = 24                    # padded pool for max8 rounds
NGR = KP // 4                # attention 512-wide groups
QSCALE = 1638.4              # fp16 key quantizer: step 6.1e-4 in [1024,2048)
QBIAS = 1024.0

_BUILD_CACHE = {}
LAST_RESULTS = None


def _build():
    nc = bacc.Bacc("TRN2", target_bir_lowering=False, debug=False,
                   enable_asserts=True, num_devices=NCORES)

    tokidx = nc.dram_tensor("tokidx", [NTOK, 1], DT.int32, kind="ExternalInput").ap()
    tok_own = nc.dram_tensor("tok_own", [TPC, 1], DT.int32, kind="ExternalInput").ap()
    table = nc.dram_tensor("table", [V, E], DT.float32, kind="ExternalInput").ap()
    tabTb = nc.dram_tensor("tabTb", [E, VS], DT.bfloat16, kind="ExternalInput").ap()
    amat = nc.dram_tensor("amat", [E, E], DT.float32, kind="ExternalInput").ap()
    bvec = nc.dram_tensor("bvec", [E, 1], DT.float32, kind="ExternalInput").ap()
    out = nc.dram_tensor("out", [TPC, E], DT.float32, kind="ExternalOutput").ap()

    with tile.TileContext(nc) as tc:
        with tc.tile_pool(name="const", bufs=1) as cpool, \
             tc.tile_pool(name="big", bufs=1) as big, \
             tc.tile_pool(name="work", bufs=2) as work, \
             tc.tile_pool(name="ps", bufs=2, space="PSUM") as ps, \
             tc.tile_pool(name="dram", bufs=1, space="DRAM") as dram:

            # ---------------- streamed strips (bf16, full residency) -----
            strip = [big.tile([P, VS], DT.bfloat16, tag=f"strip{kb}",
                              name=f"strip{kb}") for kb in range(2)]
            for g in range(NGRP):
                go, gw = GRP_OFF[g], GRP_W[g]
                for kb in range(2):
                    nc.sync.dma_start(out=strip[kb][:, go:go + gw],
                                      in_=tabTb[kb * P:(kb + 1) * P, go:go + gw])

            # ---------------- constants ----------------
            ident = cpool.tile([P, P], DT.float32, tag="ident")
            make_identity(nc, ident)

            # keys tiles: lo halves = slot iota (persist), hi = fp16 keys
            keysT = [big.tile([P, KEYW], DT.uint32, tag=f"keys{r}",
                              name=f"keys{r}") for r in range(2)]
            nc.gpsimd.iota(keysT[0], pattern=[[1, KEYW]], base=0,
                           channel_multiplier=0)
            nc.vector.tensor_copy(keysT[1], keysT[0])

            def const_col(name, val):
                t = cpool.tile([P, 1], DT.uint32, tag=name, name=name)
                nc.gpsimd.iota(t, pattern=[[0, 1]], base=val, channel_multiplier=0)
                return t

            c_mask16 = const_col("c_mask16", 0xFFFF)
            c_6 = const_col("c_6", 6)
            c_3 = const_col("c_3", 3)
            c_7 = const_col("c_7", 7)

            # preload the scalar-engine activation table (tanh/exp) so the
            # load doesn't serialize the tail
            warm = cpool.tile([P, 8], DT.float32, tag="warm")
            nc.vector.memset(warm, 0.0)
            nc.scalar.activation(warm, warm, ACT.Tanh)

            # ---------------- emb gather + transpose + bf16 cast ---------
            embT = [[big.tile([P, P], DT.bfloat16, tag=f"embT{kb}_{m}",
                              name=f"embT{kb}_{m}")
                     for m in range(NCORES)] for kb in range(2)]
            for m in range(NCORES):
                ti = work.tile([P, 1], DT.int32, tag="ti")
                nc.sync.dma_start(out=ti, in_=tokidx[m * P:(m + 1) * P, :])
                em = work.tile([P, E], DT.float32, tag="em")
                nc.gpsimd.indirect_dma_start(
                    out=em, out_offset=None, in_=table,
                    in_offset=bass.IndirectOffsetOnAxis(ap=ti[:, :], axis=0))
                pt = ps.tile([P, CW], DT.float32, tag="chunk")
                for kb in range(2):
                    nc.tensor.transpose(out=pt[:, kb * P:(kb + 1) * P],
                                        in_=em[:, kb * P:(kb + 1) * P],
                                        identity=ident)
                for kb in range(2):
                    nc.vector.tensor_copy(embT[kb][m], pt[:, kb * P:(kb + 1) * P])

            # own-token embeddings (fp32, for exact rescue dots)
            ti_own = cpool.tile([P, 1], DT.int32, tag="ti_own")
            nc.sync.dma_start(out=ti_own, in_=tok_own)
            emb_own = cpool.tile([P, E], DT.float32, tag="emb_own")
            nc.gpsimd.indirect_dma_start(
                out=emb_own, out_offset=None, in_=table,
                in_offset=bass.IndirectOffsetOnAxis(ap=ti_own[:, :], axis=0))

            # ---------------- small attention weights (f32r) -------------
            a_r = []
            for kb in range(2):
                t0 = work.tile([P, E], DT.float32, tag="aw")
                nc.sync.dma_start(out=t0, in_=amat[kb * P:(kb + 1) * P, :])
                t = cpool.tile([P, E], DT.float32r, tag=f"ar{kb}", name=f"ar{kb}")
                nc.vector.tensor_copy(t, t0)
                a_r.append(t)
            b_r = []
            for kb in range(2):
                t0 = work.tile([P, 1], DT.float32, tag="bw")
                nc.sync.dma_start(out=t0, in_=bvec[kb * P:(kb + 1) * P, :])
                t = cpool.tile([P, 1], DT.float32r, tag=f"br{kb}", name=f"br{kb}")
                nc.vector.tensor_copy(t, t0)
                b_r.append(t)

            # ---------------- a2a bounce buffers ----------------
            bounce = [dram.tile([NCORES, TPC, NCP], DT.float32, tag=f"bounce{g}",
                                name=f"bounce{g}")
                      for g in range(NGRP)]
            agg = [dram.tile([NCORES * TPC * NCP, 1], DT.float32, tag=f"agg{g}",
                             name=f"agg{g}")
                   for g in range(NGRP)]
            scd = dram.tile([1, TPC * KP], DT.float32, tag="scd")

            vals = cpool.tile([P, MERGEW], DT.float32, tag="vals")

            def load_vals(g):
                # vals[p, g*64 + c*8 + s] = agg[g][(c, p, s)]
                agg_v = agg[g][:, :].rearrange("(c p s) o -> c p (s o)",
                                               c=NCORES, p=TPC).transpose([1, 0, 2])
                out_v = vals[:, g * 64:(g + 1) * 64].rearrange(
                    "p (c s) -> p c s", c=NCORES)
                nc.sync.dma_start(out=out_v, in_=agg_v)

            def a2a(g):
                nc.gpsimd.collective_compute(
                    "AllToAll", ALU.bypass,
                    replica_groups=[list(range(NCORES))],
                    ins=[bounce[g][:, :, :].opt()],
                    outs=[agg[g][:, :].opt()],
                )

            # ---------------- similarity + per-group packed top-8 --------
            for g in range(NGRP):
                go, gw = GRP_OFF[g], GRP_W[g]
                for m in range(NCORES):
                    if m == 1 and g >= 1:
                        a2a(g - 1)
                    if m == 5 and g >= 2:
                        load_vals(g - 2)
                    kr = keysT[m & 1]
                    pss = []
                    for ci in GROUPS[g]:
                        off, w = CHUNKS[ci]
                        pchunk = ps.tile([P, CW], DT.float32, tag="chunk",
                                         name="pchunk")
                        pss.append((pchunk, off, w))
                    for kb in range(2):
                        for pt, off, w in pss:
                            for h in range(0, w, 512):
                                hw = min(512, w - h)
                                nc.tensor.matmul(
                                    pt[:, h:h + hw], embT[kb][m],
                                    strip[kb][:, off + h:off + h + hw],
                                    start=(kb == 0), stop=(kb == 1))
                    for pt, off, w in pss:
                        lo = off - go
                        hi16 = kr.bitcast(DT.float16).rearrange(
                            "p (w two) -> p two w", two=2)[:, 1, lo:lo + w]
                        nc.scalar.activation(hi16, pt[:, :w], ACT.Copy,
                                             scale=QSCALE, bias=QBIAS)
                    cv = work.tile([P, NCP], DT.float32, tag="cv", bufs=4)
                    nc.vector.max(out=cv, in_=kr.bitcast(DT.float32)[:, :gw])
                    nc.sync.dma_start(out=bounce[g][m, :, :], in_=cv)

            a2a(NGRP - 1)
            load_vals(NGRP - 2)
            load_vals(NGRP - 1)

            # ---------------- merge: top-20 keys + positions -------------
            wk = cpool.tile([P, KPAD], DT.float32, tag="wk")
            wp = cpool.tile([P, KPAD], DT.uint32, tag="wp")
            vals2 = cpool.tile([P, MERGEW], DT.float32, tag="vals2")
            vals3 = cpool.tile([P, MERGEW], DT.float32, tag="vals3")

            # decode: key = (fp16 << 16) | slot16 ; pos = g*64 + c*8 + r
            slot = cpool.tile([P, KPAD], DT.uint32, tag="slot", name="slot")
            grp = cpool.tile([P, KPAD], DT.uint32, tag="grp", name="grp")
            csrc = cpool.tile([P, KPAD], DT.uint32, tag="csrc", name="csrc")
            gidx = cpool.tile([P, KPAD], DT.uint32, tag="gidx", name="gidx")
            t2 = cpool.tile([P, KPAD], DT.uint32, tag="t2", name="t2")
            hk = [cpool.tile([P, E], DT.float32, tag=f"h{k}", name=f"h{k}")
                  for k in range(KP)]

            def decode_and_gather(g0, g1):
                """Decode candidate slots [g0,g1) and launch their h gathers."""
                gs = slice(g0, g1)
                nc.vector.tensor_scalar(slot[:, gs], wk[:, gs].bitcast(DT.uint32),
                                        c_mask16[:, :], None,
                                        op0=ALU.bitwise_and)
                nc.vector.tensor_scalar(grp[:, gs], wp[:, gs], c_6[:, :], None,
                                        op0=ALU.logical_shift_right)
                nc.vector.tensor_scalar(csrc[:, gs], wp[:, gs], c_3[:, :], None,
                                        op0=ALU.logical_shift_right)
                nc.vector.tensor_scalar(csrc[:, gs], csrc[:, gs], c_7[:, :], None,
                                        op0=ALU.bitwise_and)
                # gidx = csrc*12500 + grp*4096 + slot (< 2^24: fp-exact)
                nc.vector.tensor_scalar(gidx[:, gs], csrc[:, gs], float(VS),
                                        None, op0=ALU.mult)
                nc.vector.tensor_scalar(t2[:, gs], grp[:, gs], 4096.0, None,
                                        op0=ALU.mult)
                nc.vector.tensor_tensor(gidx[:, gs], gidx[:, gs], t2[:, gs],
                                        op=ALU.add)
                nc.vector.tensor_tensor(gidx[:, gs], gidx[:, gs], slot[:, gs],
                                        op=ALU.add)
                for k in range(g0, min(g1, KP)):
                    nc.gpsimd.indirect_dma_start(
                        out=hk[k], out_offset=None, in_=table,
                        in_offset=bass.IndirectOffsetOnAxis(
                            ap=gidx[:, :].bitcast(DT.int32)[:, k:k + 1], axis=0))

            nc.vector.max(out=wk[:, 0:8], in_=vals)
            nc.vector.max_index(out=wp[:, 0:8], in_max=wk[:, 0:8], in_values=vals)
            nc.vector.match_replace(out=vals2, in_to_replace=wk[:, 0:8],
                                    in_values=vals, imm_value=0.0)
            decode_and_gather(0, 8)
            nc.vector.max(out=wk[:, 8:16], in_=vals2)
            nc.vector.max_index(out=wp[:, 8:16], in_max=wk[:, 8:16], in_values=vals2)
            nc.vector.match_replace(out=vals3, in_to_replace=wk[:, 8:16],
                                    in_values=vals2, imm_value=0.0)
            decode_and_gather(8, 16)
            nc.vector.max(out=wk[:, 16:24], in_=vals3)
            nc.vector.max_index(out=wp[:, 16:24], in_max=wk[:, 16:24], in_values=vals3)
            decode_and_gather(16, KP)

            # ---------------- exact rescue: fp32 dots + top-16 mask ------
            d = cpool.tile([P, KPAD], DT.float32, tag="d")
            nc.vector.memset(d[:, KP:], NEG)
            prod = cpool.tile([P, E], DT.float32, tag="prod")
            for k in range(KP):
                nc.vector.scalar_tensor_tensor(
                    prod, hk[k], 1.0, emb_own,
                    op0=ALU.mult, op1=ALU.mult, accum_out=d[:, k:k + 1])

            # 16th largest exact dot -> threshold mask
            t8a = cpool.tile([P, 8], DT.float32, tag="t8a")
            t8b = cpool.tile([P, 8], DT.float32, tag="t8b")
            d2 = cpool.tile([P, KPAD], DT.float32, tag="d2")
            nc.vector.max(out=t8a, in_=d)
            nc.vector.match_replace(out=d2, in_to_replace=t8a, in_values=d,
                                    imm_value=NEG)
            nc.vector.max(out=t8b, in_=d2)
            # maskp = (1[d >= thr16] - 1) * 1e9   (0 for kept, -1e9 for dropped)
            maskp = cpool.tile([P, KP], DT.float32, tag="maskp")
            nc.vector.tensor_scalar(maskp, d[:, :KP], t8b[:, 7:8], None,
                                    op0=ALU.is_ge)
            nc.vector.tensor_scalar(maskp, maskp, -1.0, 1.0e9,
                                    op0=ALU.add, op1=ALU.mult)

            # ---------------- attention over the 20-candidate pool -------
            # hT chunks: n = k*128 + t, grouped 4 k's per 512-wide chunk
            for gA in range(NGR):
                ptt = ps.tile([P, CW], DT.float32, tag="chunk")
                for kk in range(4):
                    k = gA * 4 + kk
                    for kb in range(2):
                        nc.tensor.transpose(
                            out=ptt[:, (kb * 4 + kk) * P:(kb * 4 + kk + 1) * P],
                            in_=hk[k][:, kb * P:(kb + 1) * P],
                            identity=ident)
                hTs = [work.tile([P, 512], DT.float32r, tag=f"hTs{kb}",
                                 name=f"hTs{kb}")
                       for kb in range(2)]
                for kb in range(2):
                    for kk in range(4):
                        nc.vector.tensor_copy(
                            hTs[kb][:, kk * P:(kk + 1) * P],
                            ptt[:, (kb * 4 + kk) * P:(kb * 4 + kk + 1) * P])
                tanhTs = [work.tile([P, 512], DT.float32r, tag=f"tanhTs{eb}",
                                    name=f"tanhTs{eb}")
                          for eb in range(2)]
                pta = ps.tile([P, CW], DT.float32, tag="chunk")
                for eb in range(2):
                    for kb in range(2):
                        nc.tensor.matmul(pta[:, eb * 512:eb * 512 + 512],
                                         a_r[kb][:, eb * P:(eb + 1) * P],
                                         hTs[kb], start=(kb == 0), stop=(kb == 1))
                    nc.scalar.activation(tanhTs[eb], pta[:, eb * 512:eb * 512 + 512],
                                         ACT.Tanh)
                psc = ps.tile([P, CW], DT.float32, tag="chunk")
                for eb in range(2):
                    nc.tensor.matmul(psc[:1, :512], b_r[eb], tanhTs[eb],
                                     start=(eb == 0), stop=(eb == 1))
                scs = work.tile([1, 512], DT.float32, tag="scs")
                nc.vector.tensor_copy(scs, psc[:1, :512])
                nc.sync.dma_start(out=scd[:, gA * 512:(gA + 1) * 512], in_=scs)

            # scores [t, k] <- scd[k*128 + t]
            sct = cpool.tile([P, KP], DT.float32, tag="sct")
            for gA in range(NGR):
                nc.sync.dma_start(
                    out=sct[:, gA * 4:(gA + 1) * 4],
                    in_=scd[:, gA * 512:(gA + 1) * 512].rearrange(
                        "o (k t) -> (o t) k", t=TPC))

            # masked softmax over k
            nc.vector.tensor_tensor(sct, sct, maskp, op=ALU.add)
            mx = cpool.tile([P, 1], DT.float32, tag="mx")
            nc.vector.reduce_max(mx, sct, axis=mybir.AxisListType.X)
            negmx = cpool.tile([P, 1], DT.float32, tag="negmx")
            nc.vector.tensor_scalar(negmx, mx, -1.0, None, op0=ALU.mult)
            ex = cpool.tile([P, KP], DT.float32, tag="ex")
            nc.scalar.activation(ex, sct, ACT.Exp, bias=negmx[:, :], scale=1.0)
            sm = cpool.tile([P, 1], DT.float32, tag="sm")
            nc.vector.reduce_sum(sm, ex, axis=mybir.AxisListType.X)
            rc = cpool.tile([P, 1], DT.float32, tag="rc")
            nc.vector.reciprocal(rc, sm)
            att = cpool.tile([P, KP], DT.float32, tag="att_w")
            nc.vector.tensor_scalar(att, ex, rc[:, :], None, op0=ALU.mult)

            # out[t, e] = sum_k att[t,k] * h[t,k,e]
            acc = cpool.tile([P, E], DT.float32, tag="acc")
            accB = cpool.tile([P, E], DT.float32, tag="accB")
            nc.vector.memset(acc, 0.0)
            nc.vector.memset(accB, 0.0)
            for k in range(KP):
                if k % 2 == 0:
                    nc.vector.scalar_tensor_tensor(
                        acc, hk[k], att[:, k:k + 1], acc,
                        op0=ALU.mult, op1=ALU.add)
                else:
                    tk = cpool.tile([P, E], DT.float32, tag="term", bufs=3,
                                    name="tk")
                    nc.scalar.activation(tk, hk[k], ACT.Copy,
                                         scale=att[:, k:k + 1])
                    nc.vector.tensor_tensor(accB, accB, tk, op=ALU.add)
            nc.vector.tensor_tensor(acc, acc, accB, op=ALU.add)
            nc.sync.dma_start(out=out, in_=acc)

    nc.compile()
    return nc


def get_nc():
    if "v4" not in _BUILD_CACHE:
        _BUILD_CACHE["v4"] = _build()
    return _BUILD_CACHE["v4"]


def kernel(conceptnet_text_vec, table, a, b, topk=16, **_ignored):
    global LAST_RESULTS
    assert int(topk) == TOPK
    tok = np.asarray(conceptnet_text_vec).reshape(NTOK, 1).astype(np.int32)
    table = np.ascontiguousarray(np.asarray(table, dtype=np.float32))
    a = np.ascontiguousarray(np.asarray(a, dtype=np.float32))
    b = np.ascontiguousarray(np.asarray(b, dtype=np.float32)).reshape(E, 1)
    tabT = np.ascontiguousarray(table.T)     # [E, V]

    nc = get_nc()
    in_maps = []
    for c in range(NCORES):
        in_maps.append({
            "tokidx": tok,
            "tok_own": np.ascontiguousarray(tok[c * TPC:(c + 1) * TPC]),
            "table": table,
            "tabTb": np.ascontiguousarray(
                tabT[:, c * VS:(c + 1) * VS]).astype(ml_dtypes.bfloat16),
            "amat": a,
            "bvec": b,
        })
    trace = bool(int(os.environ.get("CN_TRACE", "0")))
    res = bass_utils.run_bass_kernel_spmd(nc, in_maps, core_ids=list(range(NCORES)),
                                          trace=trace)
    LAST_RESULTS = res
    outp = np.concatenate([res.results[c]["out"] for c in range(NCORES)], axis=0)
    return outp.reshape(B, L, E)


# revision 6
# speedup vs baseline: 1.5903x; 1.1023x over previous
"""ConceptNet encoder kernel for 8 Trainium2 NeuronCores (Bass/Tile).

Reference computation:
    emb    = table[tok]                      # [1024, 256]
    logits = emb @ table.T                   # [1024, 100000]
    idx    = top16(softmax(logits))          # softmax monotonic -> top16(logits)
    h      = table[idx]                      # [1024, 16, 256]
    e      = tanh(h @ a) @ b                 # [1024, 16]
    out    = softmax(e) @ h                  # [1024, 256]

Distribution: vocab sharded 8 ways (12500 rows/core), all 1024 tokens
scored on every core, token-sharded merge/attention (128 tokens/core).

The similarity matmul runs in bf16 (logit err sigma ~1e-4, far below
top-16 gaps ~8e-4). Selection: the scalar engine quantizes each PSUM
chunk straight into the HIGH fp16 halves of pre-iota'd uint32 words
(one strided activation per chunk) so each word is
(fp16(QSCALE*logit+QBIAS) << 16) | slot16 — totally ordered as fp32
with the slot as tiebreak.  One DVE MAX8 per 4096-wide group then
yields the top-8 (value+slot packed) with zero extra passes.  An early
barrier collective absorbs inter-core launch skew while compute runs;
four AllToAll's (one per vocab group, 8 cands each) then pipeline with
compute.  Each core merges a 256-candidate pool for its own 128
tokens, takes top-20, re-scores the pool exactly in fp32 (rescue),
and runs masked-softmax attention (bf16 matmuls, streaming phases
overlapped with the h-gathers) so exactly the true top-16 get weight.

kernel(**inputs) takes FULL unsharded inputs, returns FULL [4,256,256] output.
Self-contained: hardcodes all shapes; imports only the system concourse repo.
"""
import os
import sys

if "/opt/trn_rl_repo" not in sys.path:
    sys.path.insert(0, "/opt/trn_rl_repo")

import numpy as np
import ml_dtypes

import concourse.bass as bass
import concourse.bacc as bacc
import concourse.mybir as mybir
import concourse.tile as tile
from concourse import bass_utils
from concourse.masks import make_identity

DT = mybir.dt
ALU = mybir.AluOpType
ACT = mybir.ActivationFunctionType

B, L, V, E, TOPK = 4, 256, 100000, 256, 16
NCORES = 8
NTOK = B * L                 # 1024
TPC = NTOK // NCORES         # 128 tokens per core (merge/attention shard)
VS = V // NCORES             # 12500 vocab rows per core
P = 128
NEG = -3.0e38

CW = 2048                    # psum chunk width (4 banks)
# chunk (offset, width) within the shard; groups of chunks share one a2a
CHUNKS = [(0, 2048), (2048, 2048), (4096, 2048), (6144, 2048),
          (8192, 2048), (10240, 2048), (12288, 212)]
GROUPS = [[0, 1], [2, 3], [4, 5], [6]]
GRP_OFF = [0, 4096, 8192, 12288]
GRP_W = [4096, 4096, 4096, 212]
NGRP = 4
KEYW = 4096                  # keys tile width
NCP = 8                      # candidates per (core, group)
MERGEW = NGRP * NCORES * NCP  # 256
KP = 20                      # rescue pool size per token
KPAD = 24                    # padded pool for max8 rounds
NGR = KP // 4                # attention 512-wide groups
QSCALE = 1638.4              # fp16 key quantizer: step 6.1e-4 in [1024,2048)
QBIAS = 1024.0

_BUILD_CACHE = {}
LAST_RESULTS = None


def _build():
    nc = bacc.Bacc("TRN2", target_bir_lowering=False, debug=False,
                   enable_asserts=True, num_devices=NCORES)

    tokidx = nc.dram_tensor("tokidx", [NTOK, 1], DT.int32, kind="ExternalInput").ap()
    tok_own = nc.dram_tensor("tok_own", [TPC, 1], DT.int32, kind="ExternalInput").ap()
    table = nc.dram_tensor("table", [V, E], DT.float32, kind="ExternalInput").ap()
    tabTb = nc.dram_tensor("tabTb", [E, VS], DT.bfloat16, kind="ExternalInput").ap()
    amat = nc.dram_tensor("amat", [E, E], DT.float32, kind="ExternalInput").ap()
    bvec = nc.dram_tensor("bvec", [E, 1], DT.float32, kind="ExternalInput").ap()
    out = nc.dram_tensor("out", [TPC, E], DT.float32, kind="ExternalOutput").ap()

    with tile.TileContext(nc) as tc:
        with tc.tile_pool(name="const", bufs=1) as cpool, \
             tc.tile_pool(name="big", bufs=1) as big, \
             tc.tile_pool(name="work", bufs=2) as work, \
             tc.tile_pool(name="ps", bufs=2, space="PSUM") as ps, \
             tc.tile_pool(name="dram", bufs=1, space="DRAM") as dram:

            # ---------------- token index DMAs first (tiny, unblock gathers)
            tis = []
            for m in range(NCORES):
                ti = work.tile([P, 1], DT.int32, tag="ti", bufs=8, name="ti")
                nc.sync.dma_start(out=ti, in_=tokidx[m * P:(m + 1) * P, :])
                tis.append(ti)
            ti_own = cpool.tile([P, 1], DT.int32, tag="ti_own")
            nc.sync.dma_start(out=ti_own, in_=tok_own)

            # ---------------- strips (bf16, full residency), group order --
            strip = [big.tile([P, VS], DT.bfloat16, tag=f"strip{kb}",
                              name=f"strip{kb}") for kb in range(2)]
            for g in range(NGRP):
                go, gw = GRP_OFF[g], GRP_W[g]
                for kb in range(2):
                    nc.sync.dma_start(out=strip[kb][:, go:go + gw],
                                      in_=tabTb[kb * P:(kb + 1) * P, go:go + gw])

            # ---------------- identity + first gathers on gpsimd ---------
            ident = cpool.tile([P, P], DT.float32, tag="ident")
            make_identity(nc, ident)

            embT = [[big.tile([P, P], DT.bfloat16, tag=f"embT{kb}_{m}",
                              name=f"embT{kb}_{m}")
                     for m in range(NCORES)] for kb in range(2)]
            em_tiles = []
            for m in range(NCORES):
                em = work.tile([P, E], DT.float32, tag="em", bufs=8, name="em")
                em_tiles.append(em)

            def gather_em(m):
                nc.gpsimd.indirect_dma_start(
                    out=em_tiles[m], out_offset=None, in_=table,
                    in_offset=bass.IndirectOffsetOnAxis(ap=tis[m][:, :], axis=0))

            gather_em(0)
            gather_em(1)

            # keys tiles: lo halves = slot iota (persist), hi = fp16 keys
            keysT = [big.tile([P, KEYW], DT.uint32, tag=f"keys{r}",
                              name=f"keys{r}") for r in range(2)]
            nc.gpsimd.iota(keysT[0], pattern=[[1, KEYW]], base=0,
                           channel_multiplier=0)
            nc.vector.tensor_copy(keysT[1], keysT[0])

            for m in range(2, NCORES):
                gather_em(m)
            emb_own = cpool.tile([P, E], DT.float32, tag="emb_own")
            nc.gpsimd.indirect_dma_start(
                out=emb_own, out_offset=None, in_=table,
                in_offset=bass.IndirectOffsetOnAxis(ap=ti_own[:, :], axis=0))

            def const_col(name, val):
                t = cpool.tile([P, 1], DT.uint32, tag=name, name=name)
                nc.gpsimd.iota(t, pattern=[[0, 1]], base=val, channel_multiplier=0)
                return t

            c_mask16 = const_col("c_mask16", 0xFFFF)
            c_6 = const_col("c_6", 6)
            c_3 = const_col("c_3", 3)
            c_7 = const_col("c_7", 7)

            # barrier: absorb inter-core launch skew while compute proceeds
            bar = dram.tile([NCORES, 1], DT.float32, tag="bar", name="bar")
            bar2 = dram.tile([NCORES, 1], DT.float32, tag="bar2", name="bar2")
            nc.gpsimd.collective_compute(
                "AllToAll", ALU.bypass,
                replica_groups=[list(range(NCORES))],
                ins=[bar[:, :].opt()], outs=[bar2[:, :].opt()])

            # preload the scalar-engine activation table (tanh/exp) so the
            # load doesn't serialize the tail
            warm = cpool.tile([P, 8], DT.float32, tag="warm")
            nc.vector.memset(warm, 0.0)
            nc.scalar.activation(warm, warm, ACT.Tanh)

            # ---------------- transposes + bf16 casts of emb --------------
            for m in range(NCORES):
                pt = ps.tile([P, CW], DT.float32, tag="chunk", name="pt")
                for kb in range(2):
                    nc.tensor.transpose(out=pt[:, kb * P:(kb + 1) * P],
                                        in_=em_tiles[m][:, kb * P:(kb + 1) * P],
                                        identity=ident)
                for kb in range(2):
                    nc.vector.tensor_copy(embT[kb][m], pt[:, kb * P:(kb + 1) * P])

            # ---------------- small attention weights (bf16) --------------
            a_b = []
            for kb in range(2):
                t0 = work.tile([P, E], DT.float32, tag="aw")
                nc.sync.dma_start(out=t0, in_=amat[kb * P:(kb + 1) * P, :])
                t = cpool.tile([P, E], DT.bfloat16, tag=f"ab{kb}", name=f"ab{kb}")
                nc.vector.tensor_copy(t, t0)
                a_b.append(t)
            b_b = []
            for kb in range(2):
                t0 = work.tile([P, 1], DT.float32, tag="bw")
                nc.sync.dma_start(out=t0, in_=bvec[kb * P:(kb + 1) * P, :])
                t = cpool.tile([P, 1], DT.bfloat16, tag=f"bb{kb}", name=f"bb{kb}")
                nc.vector.tensor_copy(t, t0)
                b_b.append(t)

            # ---------------- a2a bounce buffers ----------------
            bounce = [dram.tile([NCORES, TPC, NCP], DT.float32, tag=f"bounce{g}",
                                name=f"bounce{g}")
                      for g in range(NGRP)]
            agg = [dram.tile([NCORES * TPC * NCP, 1], DT.float32, tag=f"agg{g}",
                             name=f"agg{g}")
                   for g in range(NGRP)]
            scd = dram.tile([1, TPC * KP], DT.float32, tag="scd")

            vals = cpool.tile([P, MERGEW], DT.float32, tag="vals")

            def load_vals(g):
                # vals[p, g*64 + c*8 + s] = agg[g][(c, p, s)]
                agg_v = agg[g][:, :].rearrange("(c p s) o -> c p (s o)",
                                               c=NCORES, p=TPC).transpose([1, 0, 2])
                out_v = vals[:, g * 64:(g + 1) * 64].rearrange(
                    "p (c s) -> p c s", c=NCORES)
                nc.sync.dma_start(out=out_v, in_=agg_v)

            def a2a(g):
                nc.gpsimd.collective_compute(
                    "AllToAll", ALU.bypass,
                    replica_groups=[list(range(NCORES))],
                    ins=[bounce[g][:, :, :].opt()],
                    outs=[agg[g][:, :].opt()],
                )

            # ---------------- similarity + per-group packed top-8 --------
            for g in range(NGRP):
                go, gw = GRP_OFF[g], GRP_W[g]
                for m in range(NCORES):
                    if m == 1 and g >= 1:
                        a2a(g - 1)
                    if m == 5 and g >= 2:
                        load_vals(g - 2)
                    kr = keysT[m & 1]
                    pss = []
                    for ci in GROUPS[g]:
                        off, w = CHUNKS[ci]
                        pchunk = ps.tile([P, CW], DT.float32, tag="chunk",
                                         name="pchunk")
                        pss.append((pchunk, off, w))
                    for kb in range(2):
                        for pt, off, w in pss:
                            for h in range(0, w, 512):
                                hw = min(512, w - h)
                                nc.tensor.matmul(
                                    pt[:, h:h + hw], embT[kb][m],
                                    strip[kb][:, off + h:off + h + hw],
                                    start=(kb == 0), stop=(kb == 1))
                    for pt, off, w in pss:
                        lo = off - go
                        hi16 = kr.bitcast(DT.float16).rearrange(
                            "p (w two) -> p two w", two=2)[:, 1, lo:lo + w]
                        nc.scalar.activation(hi16, pt[:, :w], ACT.Copy,
                                             scale=QSCALE, bias=QBIAS)
                    cv = work.tile([P, NCP], DT.float32, tag="cv", bufs=4)
                    nc.vector.max(out=cv, in_=kr.bitcast(DT.float32)[:, :gw])
                    nc.sync.dma_start(out=bounce[g][m, :, :], in_=cv)

            a2a(NGRP - 1)
            load_vals(NGRP - 2)
            load_vals(NGRP - 1)

            # ---------------- merge: top-20 keys + positions -------------
            wk = cpool.tile([P, KPAD], DT.float32, tag="wk")
            wp = cpool.tile([P, KPAD], DT.uint32, tag="wp")
            vals2 = cpool.tile([P, MERGEW], DT.float32, tag="vals2")
            vals3 = cpool.tile([P, MERGEW], DT.float32, tag="vals3")

            # decode: key = (fp16 << 16) | slot16 ; pos = g*64 + c*8 + r
            slot = cpool.tile([P, KPAD], DT.uint32, tag="slot", name="slot")
            grp = cpool.tile([P, KPAD], DT.uint32, tag="grp", name="grp")
            csrc = cpool.tile([P, KPAD], DT.uint32, tag="csrc", name="csrc")
            gidx = cpool.tile([P, KPAD], DT.uint32, tag="gidx", name="gidx")
            t2 = cpool.tile([P, KPAD], DT.uint32, tag="t2", name="t2")
            hk = [cpool.tile([P, E], DT.float32, tag=f"h{k}", name=f"h{k}")
                  for k in range(KP)]
            # persistent attention tiles (bf16)
            hTs = [[big.tile([P, 512], DT.bfloat16, tag=f"hTs{gA}_{kb}",
                             name=f"hTs{gA}_{kb}") for kb in range(2)]
                   for gA in range(NGR)]
            tanhTs = [[big.tile([P, 512], DT.bfloat16, tag=f"tanhTs{gA}_{eb}",
                                name=f"tanhTs{gA}_{eb}") for eb in range(2)]
                      for gA in range(NGR)]

            def decode_and_gather(g0, g1):
                """Decode candidate slots [g0,g1) and launch their h gathers."""
                gs = slice(g0, g1)
                nc.vector.tensor_scalar(slot[:, gs], wk[:, gs].bitcast(DT.uint32),
                                        c_mask16[:, :], None,
                                        op0=ALU.bitwise_and)
                nc.vector.tensor_scalar(grp[:, gs], wp[:, gs], c_6[:, :], None,
                                        op0=ALU.logical_shift_right)
                nc.vector.tensor_scalar(csrc[:, gs], wp[:, gs], c_3[:, :], None,
                                        op0=ALU.logical_shift_right)
                nc.vector.tensor_scalar(csrc[:, gs], csrc[:, gs], c_7[:, :], None,
                                        op0=ALU.bitwise_and)
                # gidx = csrc*12500 + grp*4096 + slot (< 2^24: fp-exact)
                nc.vector.tensor_scalar(gidx[:, gs], csrc[:, gs], float(VS),
                                        None, op0=ALU.mult)
                nc.vector.tensor_scalar(t2[:, gs], grp[:, gs], 4096.0, None,
                                        op0=ALU.mult)
                nc.vector.tensor_tensor(gidx[:, gs], gidx[:, gs], t2[:, gs],
                                        op=ALU.add)
                nc.vector.tensor_tensor(gidx[:, gs], gidx[:, gs], slot[:, gs],
                                        op=ALU.add)
                for k in range(g0, min(g1, KP)):
                    nc.gpsimd.indirect_dma_start(
                        out=hk[k], out_offset=None, in_=table,
                        in_offset=bass.IndirectOffsetOnAxis(
                            ap=gidx[:, :].bitcast(DT.int32)[:, k:k + 1], axis=0))

            d = cpool.tile([P, KPAD], DT.float32, tag="d")
            nc.vector.memset(d[:, KP:], NEG)
            prod = cpool.tile([P, E], DT.float32, tag="prod", bufs=2)

            def dots(k0, k1):
                # exact fp32 rescue dots on DVE
                for k in range(k0, min(k1, KP)):
                    nc.vector.scalar_tensor_tensor(
                        prod, hk[k], 1.0, emb_own,
                        op0=ALU.mult, op1=ALU.mult, accum_out=d[:, k:k + 1])

            def transpose_h(k0, k1):
                # transpose gathered h rows into psum, cast to bf16 hTs
                ptt = ps.tile([P, CW], DT.float32, tag="chunk", name="ptt")
                for k in range(k0, min(k1, KP)):
                    j = k - k0
                    for kb in range(2):
                        nc.tensor.transpose(
                            out=ptt[:, (j * 2 + kb) * P:(j * 2 + kb + 1) * P],
                            in_=hk[k][:, kb * P:(kb + 1) * P],
                            identity=ident)
                for k in range(k0, min(k1, KP)):
                    j = k - k0
                    gA, kk = k // 4, k % 4
                    # split casts scalar/DVE for balance
                    nc.vector.tensor_copy(
                        hTs[gA][0][:, kk * P:(kk + 1) * P],
                        ptt[:, (j * 2) * P:(j * 2 + 1) * P])
                    nc.scalar.activation(
                        hTs[gA][1][:, kk * P:(kk + 1) * P],
                        ptt[:, (j * 2 + 1) * P:(j * 2 + 2) * P], ACT.Copy)

            def attn_group(gA):
                pta = ps.tile([P, CW], DT.float32, tag="chunk", name="pta")
                for eb in range(2):
                    for kb in range(2):
                        nc.tensor.matmul(pta[:, eb * 512:eb * 512 + 512],
                                         a_b[kb][:, eb * P:(eb + 1) * P],
                                         hTs[gA][kb], start=(kb == 0),
                                         stop=(kb == 1))
                    nc.scalar.activation(tanhTs[gA][eb],
                                         pta[:, eb * 512:eb * 512 + 512],
                                         ACT.Tanh)
                psc = ps.tile([P, CW], DT.float32, tag="chunk", name="psc")
                for eb in range(2):
                    nc.tensor.matmul(psc[:1, :512], b_b[eb], tanhTs[gA][eb],
                                     start=(eb == 0), stop=(eb == 1))
                scs = work.tile([1, 512], DT.float32, tag="scs", bufs=3)
                nc.vector.tensor_copy(scs, psc[:1, :512])
                nc.sync.dma_start(out=scd[:, gA * 512:(gA + 1) * 512], in_=scs)

            nc.vector.max(out=wk[:, 0:8], in_=vals)
            nc.vector.max_index(out=wp[:, 0:8], in_max=wk[:, 0:8], in_values=vals)
            nc.vector.match_replace(out=vals2, in_to_replace=wk[:, 0:8],
                                    in_values=vals, imm_value=0.0)
            decode_and_gather(0, 8)
            nc.vector.max(out=wk[:, 8:16], in_=vals2)
            nc.vector.max_index(out=wp[:, 8:16], in_max=wk[:, 8:16], in_values=vals2)
            nc.vector.match_replace(out=vals3, in_to_replace=wk[:, 8:16],
                                    in_values=vals2, imm_value=0.0)
            decode_and_gather(8, 16)
            nc.vector.max(out=wk[:, 16:24], in_=vals3)
            nc.vector.max_index(out=wp[:, 16:24], in_max=wk[:, 16:24], in_values=vals3)
            decode_and_gather(16, KP)

            dots(0, 8)
            transpose_h(0, 8)
            attn_group(0)
            attn_group(1)
            dots(8, 16)
            transpose_h(8, 16)
            attn_group(2)
            attn_group(3)
            dots(16, KP)
            transpose_h(16, KP)
            attn_group(4)

            # 16th largest exact dot -> threshold mask
            t8a = cpool.tile([P, 8], DT.float32, tag="t8a")
            t8b = cpool.tile([P, 8], DT.float32, tag="t8b")
            d2 = cpool.tile([P, KPAD], DT.float32, tag="d2")
            nc.vector.max(out=t8a, in_=d)
            nc.vector.match_replace(out=d2, in_to_replace=t8a, in_values=d,
                                    imm_value=NEG)
            nc.vector.max(out=t8b, in_=d2)
            # maskp = (1[d >= thr16] - 1) * 1e9   (0 for kept, -1e9 for dropped)
            maskp = cpool.tile([P, KP], DT.float32, tag="maskp")
            nc.vector.tensor_scalar(maskp, d[:, :KP], t8b[:, 7:8], None,
                                    op0=ALU.is_ge)
            nc.vector.tensor_scalar(maskp, maskp, -1.0, 1.0e9,
                                    op0=ALU.add, op1=ALU.mult)

            # scores [t, k] <- scd[k*128 + t]
            sct = cpool.tile([P, KP], DT.float32, tag="sct")
            for gA in range(NGR):
                nc.sync.dma_start(
                    out=sct[:, gA * 4:(gA + 1) * 4],
                    in_=scd[:, gA * 512:(gA + 1) * 512].rearrange(
                        "o (k t) -> (o t) k", t=TPC))

            # masked softmax over k (scores bounded: no max-sub needed)
            nc.vector.tensor_tensor(sct, sct, maskp, op=ALU.add)
            ex = cpool.tile([P, KP], DT.float32, tag="ex")
            nc.scalar.activation(ex, sct, ACT.Exp)
            sm = cpool.tile([P, 1], DT.float32, tag="sm")
            nc.vector.reduce_sum(sm, ex, axis=mybir.AxisListType.X)
            rc = cpool.tile([P, 1], DT.float32, tag="rc")
            nc.vector.reciprocal(rc, sm)
            att = cpool.tile([P, KP], DT.float32, tag="att_w")
            nc.vector.tensor_scalar(att, ex, rc[:, :], None, op0=ALU.mult)

            # out[t, e] = sum_k att[t,k] * h[t,k,e] — 2 DVE chains + scalar
            acc = cpool.tile([P, E], DT.float32, tag="acc")
            acc2 = cpool.tile([P, E], DT.float32, tag="acc2")
            accB = cpool.tile([P, E], DT.float32, tag="accB")
            accB2 = cpool.tile([P, E], DT.float32, tag="accB2")
            nc.vector.memset(acc, 0.0)
            nc.vector.memset(acc2, 0.0)
            nc.vector.memset(accB, 0.0)
            nc.vector.memset(accB2, 0.0)
            for k in range(KP):
                if k % 2 == 0:
                    ac = acc if (k // 2) % 2 == 0 else acc2
                    nc.vector.scalar_tensor_tensor(
                        ac, hk[k], att[:, k:k + 1], ac,
                        op0=ALU.mult, op1=ALU.add)
                else:
                    tk = cpool.tile([P, E], DT.float32, tag="term", bufs=3,
                                    name="tk")
                    nc.scalar.activation(tk, hk[k], ACT.Copy,
                                         scale=att[:, k:k + 1])
                    ab = accB if (k // 2) % 2 == 0 else accB2
                    nc.vector.tensor_tensor(ab, ab, tk, op=ALU.add)
            nc.vector.tensor_tensor(acc, acc, acc2, op=ALU.add)
            nc.vector.tensor_tensor(accB, accB, accB2, op=ALU.add)
            nc.vector.tensor_tensor(acc, acc, accB, op=ALU.add)
            nc.sync.dma_start(out=out, in_=acc)

    nc.compile()
    return nc


def get_nc():
    if "v5" not in _BUILD_CACHE:
        _BUILD_CACHE["v5"] = _build()
    return _BUILD_CACHE["v5"]


def kernel(conceptnet_text_vec, table, a, b, topk=16, **_ignored):
    global LAST_RESULTS
    assert int(topk) == TOPK
    tok = np.asarray(conceptnet_text_vec).reshape(NTOK, 1).astype(np.int32)
    table = np.ascontiguousarray(np.asarray(table, dtype=np.float32))
    a = np.ascontiguousarray(np.asarray(a, dtype=np.float32))
    b = np.ascontiguousarray(np.asarray(b, dtype=np.float32)).reshape(E, 1)
    tabT = np.ascontiguousarray(table.T)     # [E, V]

    nc = get_nc()
    in_maps = []
    for c in range(NCORES):
        in_maps.append({
            "tokidx": tok,
            "tok_own": np.ascontiguousarray(tok[c * TPC:(c + 1) * TPC]),
            "table": table,
            "tabTb": np.ascontiguousarray(
                tabT[:, c * VS:(c + 1) * VS]).astype(ml_dtypes.bfloat16),
            "amat": a,
            "bvec": b,
        })
    trace = bool(int(os.environ.get("CN_TRACE", "0")))
    res = bass_utils.run_bass_kernel_spmd(nc, in_maps, core_ids=list(range(NCORES)),
                                          trace=trace)
    LAST_RESULTS = res
    outp = np.concatenate([res.results[c]["out"] for c in range(NCORES)], axis=0)
    return outp.reshape(B, L, E)


# revision 12
# speedup vs baseline: 1.7483x; 1.0994x over previous
"""ConceptNet encoder kernel for 8 Trainium2 NeuronCores (Bass/Tile).

Reference computation:
    emb    = table[tok]                      # [1024, 256]
    logits = emb @ table.T                   # [1024, 100000]
    idx    = top16(softmax(logits))          # softmax monotonic -> top16(logits)
    h      = table[idx]                      # [1024, 16, 256]
    e      = tanh(h @ a) @ b                 # [1024, 16]
    out    = softmax(e) @ h                  # [1024, 256]

Distribution: vocab sharded 8 ways (12500 rows/core), all 1024 tokens
scored on every core, token-sharded merge/attention (128 tokens/core).

The similarity matmul runs in bf16 (logit err sigma ~1e-4, far below
top-16 gaps ~8e-4). Selection: each PSUM chunk is quantized straight
into the HIGH fp16 halves of pre-iota'd uint32 words (one strided
activation per chunk; chunk A on the scalar engine, chunk B on
gpsimd) so each word is (fp16(QSCALE*logit+QBIAS) << 16) | slot16 —
totally ordered as fp32 with the slot as tiebreak.  One DVE MAX8 per
4096-wide group then yields the top-8 (value+slot packed) with zero
extra passes.  A barrier collective gates the key-iota so all cores
enter the (scalar/DVE-paced) main loop aligned; four AllToAll's then
pipeline with compute and the final one completes in ~2us.  Each core
merges a 256-candidate pool for its own 128 tokens, takes top-20,
re-scores the pool exactly in fp32 (rescue), and runs masked-softmax
attention (bf16 matmuls, streamed right behind the h-gathers through
a ring of 1-bank PSUM tiles) so exactly the true top-16 get weight.

kernel(**inputs) takes FULL unsharded inputs, returns FULL [4,256,256] output.
Self-contained: hardcodes all shapes; imports only the system concourse repo.
"""
import os
import sys

if "/opt/trn_rl_repo" not in sys.path:
    sys.path.insert(0, "/opt/trn_rl_repo")

import numpy as np
import ml_dtypes

import concourse.bass as bass
import concourse.bacc as bacc
import concourse.mybir as mybir
import concourse.tile as tile
from concourse import bass_utils
from concourse.masks import make_identity

DT = mybir.dt
ALU = mybir.AluOpType
ACT = mybir.ActivationFunctionType

B, L, V, E, TOPK = 4, 256, 100000, 256, 16
NCORES = 8
NTOK = B * L                 # 1024
TPC = NTOK // NCORES         # 128 tokens per core (merge/attention shard)
VS = V // NCORES             # 12500 vocab rows per core
P = 128
NEG = -3.0e38

CW = 2048                    # psum chunk width (4 banks)
CHUNKS = [(0, 2048), (2048, 2048), (4096, 2048), (6144, 2048),
          (8192, 2048), (10240, 2048), (12288, 212)]
GROUPS = [[0, 1], [2, 3], [4, 5], [6]]
GRP_OFF = [0, 4096, 8192, 12288]
GRP_W = [4096, 4096, 4096, 212]
NGRP = 4
KEYW = 4096                  # keys tile width
NCP = 8                      # candidates per (core, group)
MERGEW = NGRP * NCORES * NCP  # 256
KP = 20                      # rescue pool size per token
KPAD = 24                    # padded pool for max8 rounds
NGR = KP // 4                # attention 512-wide groups
QSCALE = 1638.4              # fp16 key quantizer: step 6.1e-4 in [1024,2048)
QBIAS = 1024.0

_BUILD_CACHE = {}
LAST_RESULTS = None


def _build():
    nc = bacc.Bacc("TRN2", target_bir_lowering=False, debug=False,
                   enable_asserts=True, num_devices=NCORES)

    tokidx = nc.dram_tensor("tokidx", [NTOK, 1], DT.int32, kind="ExternalInput").ap()
    tok_own = nc.dram_tensor("tok_own", [TPC, 1], DT.int32, kind="ExternalInput").ap()
    table = nc.dram_tensor("table", [V, E], DT.float32, kind="ExternalInput").ap()
    tabTb = nc.dram_tensor("tabTb", [E, VS], DT.bfloat16, kind="ExternalInput").ap()
    amat = nc.dram_tensor("amat", [E, E], DT.float32, kind="ExternalInput").ap()
    bvec = nc.dram_tensor("bvec", [E, 1], DT.float32, kind="ExternalInput").ap()
    out = nc.dram_tensor("out", [TPC, E], DT.float32, kind="ExternalOutput").ap()

    with tile.TileContext(nc) as tc:
        with tc.tile_pool(name="const", bufs=1) as cpool, \
             tc.tile_pool(name="big", bufs=1) as big, \
             tc.tile_pool(name="work", bufs=2) as work, \
             tc.tile_pool(name="dram", bufs=1, space="DRAM") as dram:

            # ---------------- token index DMAs first (tiny) --------------
            tis = []
            for m in range(NCORES):
                ti = work.tile([P, 1], DT.int32, tag="ti", bufs=8, name="ti")
                nc.sync.dma_start(out=ti, in_=tokidx[m * P:(m + 1) * P, :])
                tis.append(ti)
            ti_own = cpool.tile([P, 1], DT.int32, tag="ti_own")
            nc.sync.dma_start(out=ti_own, in_=tok_own)

            # ---------------- strips (bf16, full residency), group order --
            strip = [big.tile([P, VS], DT.bfloat16, tag=f"strip{kb}",
                              name=f"strip{kb}") for kb in range(2)]
            for g in range(NGRP):
                go, gw = GRP_OFF[g], GRP_W[g]
                for kb in range(2):
                    nc.sync.dma_start(out=strip[kb][:, go:go + gw],
                                      in_=tabTb[kb * P:(kb + 1) * P, go:go + gw])

            # ---------------- identity + gathers on gpsimd ----------------
            ident = cpool.tile([P, P], DT.float32, tag="ident")
            make_identity(nc, ident)

            em_tiles = []
            for m in range(NCORES):
                em = work.tile([P, E], DT.float32, tag="em", bufs=8, name="em")
                em_tiles.append(em)
            for m in range(NCORES):
                nc.gpsimd.indirect_dma_start(
                    out=em_tiles[m], out_offset=None, in_=table,
                    in_offset=bass.IndirectOffsetOnAxis(ap=tis[m][:, :], axis=0))
            emb_own = cpool.tile([P, E], DT.float32, tag="emb_own")
            nc.gpsimd.indirect_dma_start(
                out=emb_own, out_offset=None, in_=table,
                in_offset=bass.IndirectOffsetOnAxis(ap=ti_own[:, :], axis=0))

            def const_col(name, val):
                t = cpool.tile([P, 1], DT.uint32, tag=name, name=name)
                nc.gpsimd.iota(t, pattern=[[0, 1]], base=val, channel_multiplier=0)
                return t

            c_mask16 = const_col("c_mask16", 0xFFFF)
            c_6 = const_col("c_6", 6)
            c_3 = const_col("c_3", 3)
            c_7 = const_col("c_7", 7)

            # barrier: aligns cores (keys iota — and so the whole
            # scalar/DVE-paced main loop — waits for it; compute queues
            # keep running on already-issued work meanwhile)
            bar = dram.tile([NCORES, 1], DT.float32, tag="bar", name="bar")
            bar2 = dram.tile([NCORES, 1], DT.float32, tag="bar2", name="bar2")
            nc.gpsimd.collective_compute(
                "AllToAll", ALU.bypass,
                replica_groups=[list(range(NCORES))],
                ins=[bar[:, :].opt()], outs=[bar2[:, :].opt()])

            # keys tiles: lo halves = slot iota (persist), hi = fp16 keys
            keysT = [big.tile([P, KEYW], DT.uint32, tag=f"keys{r}",
                              name=f"keys{r}") for r in range(2)]
            nc.gpsimd.iota(keysT[0][:, :2048], pattern=[[1, 2048]], base=0,
                           channel_multiplier=0)
            nc.vector.tensor_scalar(keysT[0][:, 2048:], keysT[0][:, :2048],
                                    2048.0, None, op0=ALU.add)
            nc.vector.tensor_copy(keysT[1], keysT[0])

            # preload the scalar-engine activation table (tanh/exp)
            warm = cpool.tile([P, 8], DT.float32, tag="warm")
            nc.vector.memset(warm, 0.0)
            nc.scalar.activation(warm, warm, ACT.Tanh)

            # ---------------- small attention weights (bf16) --------------
            a_b = []
            for kb in range(2):
                t0 = work.tile([P, E], DT.float32, tag="aw")
                nc.sync.dma_start(out=t0, in_=amat[kb * P:(kb + 1) * P, :])
                t = cpool.tile([P, E], DT.bfloat16, tag=f"ab{kb}", name=f"ab{kb}")
                nc.vector.tensor_copy(t, t0)
                a_b.append(t)
            b_b = []
            for kb in range(2):
                t0 = work.tile([P, 1], DT.float32, tag="bw")
                nc.sync.dma_start(out=t0, in_=bvec[kb * P:(kb + 1) * P, :])
                t = cpool.tile([P, 1], DT.bfloat16, tag=f"bb{kb}", name=f"bb{kb}")
                nc.vector.tensor_copy(t, t0)
                b_b.append(t)

            # ---------------- a2a bounce buffers ----------------
            bounce = [dram.tile([NCORES, TPC, NCP], DT.float32, tag=f"bounce{g}",
                                name=f"bounce{g}")
                      for g in range(NGRP)]
            agg = [dram.tile([NCORES * TPC * NCP, 1], DT.float32, tag=f"agg{g}",
                             name=f"agg{g}")
                   for g in range(NGRP)]
            scd = dram.tile([1, TPC * KP], DT.float32, tag="scd")

            vals = cpool.tile([P, MERGEW], DT.float32, tag="vals")

            def load_vals(g):
                # vals[p, g*64 + c*8 + s] = agg[g][(c, p, s)]
                agg_v = agg[g][:, :].rearrange("(c p s) o -> c p (s o)",
                                               c=NCORES, p=TPC).transpose([1, 0, 2])
                out_v = vals[:, g * 64:(g + 1) * 64].rearrange(
                    "p (c s) -> p c s", c=NCORES)
                nc.sync.dma_start(out=out_v, in_=agg_v)

            def a2a(g):
                nc.gpsimd.collective_compute(
                    "AllToAll", ALU.bypass,
                    replica_groups=[list(range(NCORES))],
                    ins=[bounce[g][:, :, :].opt()],
                    outs=[agg[g][:, :].opt()],
                )

            embT = [[big.tile([P, P], DT.bfloat16, tag=f"embT{kb}_{m}",
                              name=f"embT{kb}_{m}")
                     for m in range(NCORES)] for kb in range(2)]

            # ============ main pipeline: psum pool scope =================
            with tc.tile_pool(name="ps", bufs=2, space="PSUM") as ps:
                # transposes + bf16 casts of emb
                for m in range(NCORES):
                    pt = ps.tile([P, CW], DT.float32, tag="chunk", name="pt")
                    for kb in range(2):
                        nc.tensor.transpose(out=pt[:, kb * P:(kb + 1) * P],
                                            in_=em_tiles[m][:, kb * P:(kb + 1) * P],
                                            identity=ident)
                    for kb in range(2):
                        nc.vector.tensor_copy(embT[kb][m],
                                              pt[:, kb * P:(kb + 1) * P])

                # similarity + per-group packed top-8
                for g in range(NGRP):
                    go, gw = GRP_OFF[g], GRP_W[g]
                    for m in range(NCORES):
                        if m == 1 and g >= 1:
                            a2a(g - 1)
                        if m == 5 and g >= 2:
                            load_vals(g - 2)
                        kr = keysT[m & 1]
                        pss = []
                        for ci in GROUPS[g]:
                            off, w = CHUNKS[ci]
                            pchunk = ps.tile([P, CW], DT.float32, tag="chunk",
                                             name="pchunk")
                            pss.append((pchunk, off, w))
                        for kb in range(2):
                            for pt, off, w in pss:
                                for h in range(0, w, 512):
                                    hw = min(512, w - h)
                                    nc.tensor.matmul(
                                        pt[:, h:h + hw], embT[kb][m],
                                        strip[kb][:, off + h:off + h + hw],
                                        start=(kb == 0), stop=(kb == 1))
                        for pt, off, w in pss:
                            lo = off - go
                            hi16 = kr.bitcast(DT.float16).rearrange(
                                "p (w two) -> p two w", two=2)[:, 1, lo:lo + w]
                            nc.scalar.activation(hi16, pt[:, :w], ACT.Copy,
                                                 scale=QSCALE, bias=QBIAS)
                        cv = work.tile([P, NCP], DT.float32, tag="cv", bufs=4)
                        nc.vector.max(out=cv, in_=kr.bitcast(DT.float32)[:, :gw])
                        nc.sync.dma_start(out=bounce[g][m, :, :], in_=cv)

                a2a(NGRP - 1)
                load_vals(NGRP - 2)
                load_vals(NGRP - 1)

            # ============ merge / rescue / attention: 1-bank psum ========
            wk = cpool.tile([P, KPAD], DT.float32, tag="wk")
            wp = cpool.tile([P, KPAD], DT.uint32, tag="wp")
            vals2 = cpool.tile([P, MERGEW], DT.float32, tag="vals2")
            vals3 = cpool.tile([P, MERGEW], DT.float32, tag="vals3")
            slot = cpool.tile([P, KPAD], DT.uint32, tag="slot", name="slot")
            grp = cpool.tile([P, KPAD], DT.uint32, tag="grp", name="grp")
            csrc = cpool.tile([P, KPAD], DT.uint32, tag="csrc", name="csrc")
            gidx = cpool.tile([P, KPAD], DT.uint32, tag="gidx", name="gidx")
            t2 = cpool.tile([P, KPAD], DT.uint32, tag="t2", name="t2")
            hk = [cpool.tile([P, E], DT.float32, tag=f"h{k}", name=f"h{k}")
                  for k in range(KP)]
            hTs = [[big.tile([P, 512], DT.bfloat16, tag=f"hTs{gA}_{kb}",
                             name=f"hTs{gA}_{kb}") for kb in range(2)]
                   for gA in range(NGR)]
            tanhTs = [[big.tile([P, 512], DT.bfloat16, tag=f"tanhTs{gA}_{eb}",
                                name=f"tanhTs{gA}_{eb}") for eb in range(2)]
                      for gA in range(NGR)]
            d = cpool.tile([P, KPAD], DT.float32, tag="d")
            nc.vector.memset(d[:, KP:], NEG)
            prod = cpool.tile([P, E], DT.float32, tag="prod", bufs=2)

            with tc.tile_pool(name="ps2", bufs=1, space="PSUM") as ps2:

                def decode_and_gather(g0, g1):
                    gs = slice(g0, g1)
                    nc.vector.tensor_scalar(slot[:, gs],
                                            wk[:, gs].bitcast(DT.uint32),
                                            c_mask16[:, :], None,
                                            op0=ALU.bitwise_and)
                    nc.vector.tensor_scalar(grp[:, gs], wp[:, gs], c_6[:, :],
                                            None, op0=ALU.logical_shift_right)
                    nc.vector.tensor_scalar(csrc[:, gs], wp[:, gs], c_3[:, :],
                                            None, op0=ALU.logical_shift_right)
                    nc.vector.tensor_scalar(csrc[:, gs], csrc[:, gs], c_7[:, :],
                                            None, op0=ALU.bitwise_and)
                    nc.vector.tensor_scalar(gidx[:, gs], csrc[:, gs], float(VS),
                                            None, op0=ALU.mult)
                    nc.vector.tensor_scalar(t2[:, gs], grp[:, gs], 4096.0, None,
                                            op0=ALU.mult)
                    nc.vector.tensor_tensor(gidx[:, gs], gidx[:, gs], t2[:, gs],
                                            op=ALU.add)
                    nc.vector.tensor_tensor(gidx[:, gs], gidx[:, gs], slot[:, gs],
                                            op=ALU.add)
                    for k in range(g0, min(g1, KP)):
                        nc.gpsimd.indirect_dma_start(
                            out=hk[k], out_offset=None, in_=table,
                            in_offset=bass.IndirectOffsetOnAxis(
                                ap=gidx[:, :].bitcast(DT.int32)[:, k:k + 1],
                                axis=0))

                def dots(k0, k1):
                    for k in range(k0, min(k1, KP)):
                        nc.vector.scalar_tensor_tensor(
                            prod, hk[k], 1.0, emb_own,
                            op0=ALU.mult, op1=ALU.mult, accum_out=d[:, k:k + 1])

                def transpose_pair(k0):
                    # transpose hk[k0], hk[k0+1] into one 1-bank psum tile
                    ptt = ps2.tile([P, 512], DT.float32, tag="ptr", name="ptt",
                                   bufs=3)
                    for j in range(2):
                        k = k0 + j
                        for kb in range(2):
                            nc.tensor.transpose(
                                out=ptt[:, (j * 2 + kb) * P:(j * 2 + kb + 1) * P],
                                in_=hk[k][:, kb * P:(kb + 1) * P],
                                identity=ident)
                    for j in range(2):
                        k = k0 + j
                        gA, kk = k // 4, k % 4
                        nc.vector.tensor_copy(
                            hTs[gA][0][:, kk * P:(kk + 1) * P],
                            ptt[:, (j * 2) * P:(j * 2 + 1) * P])
                        nc.scalar.activation(
                            hTs[gA][1][:, kk * P:(kk + 1) * P],
                            ptt[:, (j * 2 + 1) * P:(j * 2 + 2) * P], ACT.Copy)

                def attn_group(gA):
                    for eb in range(2):
                        pta = ps2.tile([P, 512], DT.float32, tag="pta",
                                       name="pta", bufs=3)
                        for kb in range(2):
                            nc.tensor.matmul(pta, a_b[kb][:, eb * P:(eb + 1) * P],
                                             hTs[gA][kb], start=(kb == 0),
                                             stop=(kb == 1))
                        nc.scalar.activation(tanhTs[gA][eb], pta, ACT.Tanh)
                    psc = ps2.tile([P, 512], DT.float32, tag="psc", name="psc",
                                   bufs=2)
                    for eb in range(2):
                        nc.tensor.matmul(psc[:1, :], b_b[eb], tanhTs[gA][eb],
                                         start=(eb == 0), stop=(eb == 1))
                    scs = work.tile([1, 512], DT.float32, tag="scs", bufs=3)
                    nc.vector.tensor_copy(scs, psc[:1, :])
                    nc.sync.dma_start(out=scd[:, gA * 512:(gA + 1) * 512], in_=scs)

                nc.vector.max(out=wk[:, 0:8], in_=vals)
                nc.vector.max_index(out=wp[:, 0:8], in_max=wk[:, 0:8],
                                    in_values=vals)
                nc.vector.match_replace(out=vals2, in_to_replace=wk[:, 0:8],
                                        in_values=vals, imm_value=0.0)
                decode_and_gather(0, 8)
                nc.vector.max(out=wk[:, 8:16], in_=vals2)
                nc.vector.max_index(out=wp[:, 8:16], in_max=wk[:, 8:16],
                                    in_values=vals2)
                nc.vector.match_replace(out=vals3, in_to_replace=wk[:, 8:16],
                                        in_values=vals2, imm_value=0.0)
                decode_and_gather(8, 16)
                nc.vector.max(out=wk[:, 16:24], in_=vals3)
                nc.vector.max_index(out=wp[:, 16:24], in_max=wk[:, 16:24],
                                    in_values=vals3)
                decode_and_gather(16, KP)

                dots(0, 8)
                for k0 in (0, 2, 4, 6):
                    transpose_pair(k0)
                attn_group(0)
                attn_group(1)
                dots(8, 16)
                for k0 in (8, 10, 12, 14):
                    transpose_pair(k0)
                attn_group(2)
                attn_group(3)
                dots(16, KP)
                for k0 in (16, 18):
                    transpose_pair(k0)
                attn_group(4)

                # 16th largest exact dot -> threshold mask
                t8a = cpool.tile([P, 8], DT.float32, tag="t8a")
                t8b = cpool.tile([P, 8], DT.float32, tag="t8b")
                d2 = cpool.tile([P, KPAD], DT.float32, tag="d2")
                nc.vector.max(out=t8a, in_=d)
                nc.vector.match_replace(out=d2, in_to_replace=t8a, in_values=d,
                                        imm_value=NEG)
                nc.vector.max(out=t8b, in_=d2)
                maskp = cpool.tile([P, KP], DT.float32, tag="maskp")
                nc.vector.tensor_scalar(maskp, d[:, :KP], t8b[:, 7:8], None,
                                        op0=ALU.is_ge)
                nc.vector.tensor_scalar(maskp, maskp, -1.0, 1.0e9,
                                        op0=ALU.add, op1=ALU.mult)

                # scores [t, k] <- scd[k*128 + t]; per-group unnormalized
                # softmax-accumulate (scores bounded: no max-sub needed)
                sct = cpool.tile([P, KP], DT.float32, tag="sct")
                exv = cpool.tile([P, KP], DT.float32, tag="exv")
                acc = cpool.tile([P, E], DT.float32, tag="acc")
                acc2 = cpool.tile([P, E], DT.float32, tag="acc2")
                nc.vector.memset(acc, 0.0)
                nc.vector.memset(acc2, 0.0)
                for gA in range(NGR):
                    gs = slice(gA * 4, (gA + 1) * 4)
                    nc.sync.dma_start(
                        out=sct[:, gs],
                        in_=scd[:, gA * 512:(gA + 1) * 512].rearrange(
                            "o (k t) -> (o t) k", t=TPC))
                    nc.vector.tensor_tensor(sct[:, gs], sct[:, gs],
                                            maskp[:, gs], op=ALU.add)
                    nc.scalar.activation(exv[:, gs], sct[:, gs], ACT.Exp)
                    for k in range(gA * 4, (gA + 1) * 4):
                        ac = acc if k % 2 == 0 else acc2
                        nc.vector.scalar_tensor_tensor(
                            ac, hk[k], exv[:, k:k + 1], ac,
                            op0=ALU.mult, op1=ALU.add)
                sm = cpool.tile([P, 1], DT.float32, tag="sm")
                nc.vector.reduce_sum(sm, exv, axis=mybir.AxisListType.X)
                rc = cpool.tile([P, 1], DT.float32, tag="rc")
                nc.vector.reciprocal(rc, sm)
                nc.vector.tensor_tensor(acc, acc, acc2, op=ALU.add)
                nc.vector.tensor_scalar(acc, acc, rc[:, :], None, op0=ALU.mult)
                nc.sync.dma_start(out=out, in_=acc)

    nc.compile()
    return nc


def get_nc():
    if "v6" not in _BUILD_CACHE:
        _BUILD_CACHE["v6"] = _build()
    return _BUILD_CACHE["v6"]


def kernel(conceptnet_text_vec, table, a, b, topk=16, **_ignored):
    global LAST_RESULTS
    assert int(topk) == TOPK
    tok = np.asarray(conceptnet_text_vec).reshape(NTOK, 1).astype(np.int32)
    table = np.ascontiguousarray(np.asarray(table, dtype=np.float32))
    a = np.ascontiguousarray(np.asarray(a, dtype=np.float32))
    b = np.ascontiguousarray(np.asarray(b, dtype=np.float32)).reshape(E, 1)
    tabT = np.ascontiguousarray(table.T)     # [E, V]

    nc = get_nc()
    in_maps = []
    for c in range(NCORES):
        in_maps.append({
            "tokidx": tok,
            "tok_own": np.ascontiguousarray(tok[c * TPC:(c + 1) * TPC]),
            "table": table,
            "tabTb": np.ascontiguousarray(
                tabT[:, c * VS:(c + 1) * VS]).astype(ml_dtypes.bfloat16),
            "amat": a,
            "bvec": b,
        })
    trace = bool(int(os.environ.get("CN_TRACE", "0")))
    res = bass_utils.run_bass_kernel_spmd(nc, in_maps, core_ids=list(range(NCORES)),
                                          trace=trace)
    LAST_RESULTS = res
    outp = np.concatenate([res.results[c]["out"] for c in range(NCORES)], axis=0)
    return outp.reshape(B, L, E)
